# revision 1
# baseline (speedup 1.0000x reference)
"""ColorRandomizer Trainium2 kernel: brightness/contrast/saturation/hue on 8 cores.

Data-parallel: 4 images per core. Per image:
  ph1  x1 = min(x*bf, 1)                (DVE tensor_scalar, fp32->fp16, + free accum for mean)
  ph2  x2 = clip01(cf*x1 + (1-cf)*mean) (ACT relu affine + DVE min)
  ph3  x3 = clip01(sf*x2 + (1-sf)*gray(x2))
  ph4  HSV hue shift, reformulated:
         J = cr*H6 built w/o division via case-select (copy_predicated),
         i6 = J * exp(-ln(cr+eps)) + 6*hf,
         out_c = min(minc + cr*relu(min(|i6+a_c|,|i6+a_c-6|) - 1), maxc)
       (tent identity for HSV->RGB; no floor/mod needed for |hf|<=1/3)
Storage fp16 (validated absmax ~6.5e-3 vs fp32 reference), scalars/accums fp32.
"""
import sys

for _p in ("/opt/trn_rl_repo",):
    if _p not in sys.path:
        sys.path.append(_p)

import numpy as np
from concourse import bass, bacc, mybir, tile, bass_isa
from concourse.bass_utils import run_bass_kernel_spmd

F32 = mybir.dt.float32
F16 = mybir.dt.float16
OP = mybir.AluOpType
AF = mybir.ActivationFunctionType

NIMG = 4          # images per core
H, W = 480, 640
NPIX = H * W      # 307200
F = NPIX // 128   # 2400 free elems per partition per plane
F3 = 3 * F
GRAY_W = (0.299, 0.587, 0.114)

TRACE = False     # test.py flips this for profiling
_CACHE = {}


def _build():
    nc = bacc.Bacc(None, target_bir_lowering=False)
    x_h = nc.declare_dram_parameter("x", [NIMG, 3, H, W], F32, isOutput=False)
    fac_h = nc.declare_dram_parameter("fac", [NIMG, 8], F32, isOutput=False)
    y_h = nc.declare_dram_parameter("y", [NIMG, 3, H, W], F32, isOutput=True)

    dma = nc.sync  # HWDGE

    # activation float biases must exist as const APs
    for v in (1e-30, -1.0, 3.0, -3.0, -5.0, -7.0):
        t = nc.alloc_sbuf_tensor(f"cst-{v}", [128, 1], F32)
        nc.gpsimd.memset(t.ap(), v)
        nc.const_aps.aps[(F32, v)] = t.ap()
    nc.all_engine_barrier()

    with tile.TileContext(nc) as tc:
        with tc.tile_pool(name="p", bufs=1) as pool:
            # broadcast per-image factors to all partitions once
            fac1 = pool.tile([1, NIMG * 8], F32)
            dma.dma_start(fac1[:], fac_h[:].flatten()[None, :])
            facb = pool.tile([128, NIMG * 8], F32)
            nc.gpsimd.partition_broadcast(facb[:], fac1[:], channels=128)

            def col(i, k):
                return facb[:, i * 8 + k : i * 8 + k + 1]

            for i in range(NIMG):
                # ---- load ----
                xin = pool.tile([128, F3], F32, tag="io", bufs=2)
                for c in range(3):
                    dma.dma_start(
                        xin[:, c * F : (c + 1) * F],
                        x_h[i, c].flatten().rearrange("(p f) -> p f", p=128),
                    )

                # ---- ph1: brightness + per-channel sums ----
                rgb = pool.tile([128, F3], F16, tag="rgb", bufs=2)
                sums = pool.tile([128, 4], F32, tag="sums", bufs=2)
                jt = pool.tile([128, F3], F16, tag="jt")
                for c in range(3):
                    nc.vector.tensor_scalar(
                        rgb[:, c * F : (c + 1) * F],
                        xin[:, c * F : (c + 1) * F],
                        col(i, 0), 1.0, OP.mult, OP.min,
                    )
                    # per-channel sums for the contrast mean (ACT accum; DVE
                    # tensor_scalar accum_out is broken on HW)
                    nc.scalar.activation(
                        jt[:, c * F : (c + 1) * F],
                        rgb[:, c * F : (c + 1) * F],
                        AF.Identity, bias=0.0, scale=1.0,
                        accum_out=sums[:, c : c + 1],
                    )
                # weighted per-partition sum -> all-core scalar -> delta
                ws = pool.tile([128, 1], F32, tag="ws", bufs=2)
                nc.vector.tensor_scalar(ws[:], sums[:, 0:1], GRAY_W[0], None, OP.mult)
                ws2 = pool.tile([128, 1], F32, tag="ws2", bufs=2)
                nc.vector.scalar_tensor_tensor(ws2[:], sums[:, 1:2], GRAY_W[1], ws[:], OP.mult, OP.add)
                ws3 = pool.tile([128, 1], F32, tag="ws3", bufs=2)
                nc.vector.scalar_tensor_tensor(ws3[:], sums[:, 2:3], GRAY_W[2], ws2[:], OP.mult, OP.add)
                ssum = pool.tile([128, 1], F32, tag="ssum", bufs=2)
                nc.gpsimd.partition_all_reduce(ssum[:], ws3[:], 128, bass_isa.ReduceOp.add)
                delta = pool.tile([128, 1], F32, tag="delta", bufs=2)
                nc.vector.tensor_tensor(delta[:], ssum[:], col(i, 2), OP.mult)

                # ---- ph2: contrast (upper clip fused into ph3 consumers) ----
                ya = pool.tile([128, F3], F16, tag="ya", bufs=2)
                nc.scalar.activation(rgb[:], rgb[:], AF.Relu, bias=delta[:], scale=col(i, 1))

                # ---- ph3: saturation;  gs = (1-sf)*gray(x2) built in ya ----
                # each consumer applies the pending "min 1" via fused 2-scalar TS
                nc.vector.tensor_scalar(ya[:, 0:F], rgb[:, 0:F], 1.0, col(i, 4), OP.min, OP.mult)
                nc.vector.tensor_scalar(ya[:, F:2 * F], rgb[:, F:2 * F], 1.0, col(i, 5), OP.min, OP.mult)
                nc.vector.tensor_tensor(ya[:, 2 * F:3 * F], ya[:, F:2 * F], ya[:, 0:F], OP.add)
                nc.vector.tensor_scalar(ya[:, 0:F], rgb[:, 2 * F:3 * F], 1.0, col(i, 6), OP.min, OP.mult)
                nc.vector.tensor_tensor(ya[:, F:2 * F], ya[:, 0:F], ya[:, 2 * F:3 * F], OP.add)
                y3 = pool.tile([128, F3], F16, tag="y3")
                nc.vector.tensor_scalar(y3[:], rgb[:], 1.0, col(i, 3), OP.min, OP.mult)
                gsb = ya[:, F:2 * F][:, None, :].broadcast_to([128, 3, F])
                nc.vector.tensor_tensor(
                    jt[:].rearrange("p (c f) -> p c f", c=3),
                    y3[:].rearrange("p (c f) -> p c f", c=3),
                    gsb, OP.add,
                )
                nc.vector.tensor_scalar(rgb[:], jt[:], 0.0, 1.0, OP.max, OP.min)

                # ---- ph4: hue ----
                # ya: [0:F]=maxc  [F:2F]=minc  [2F:3F]=cr   (2F used as scratch first)
                nc.vector.tensor_tensor(ya[:, 2 * F:3 * F], rgb[:, 0:F], rgb[:, F:2 * F], OP.max)
                nc.vector.tensor_tensor(ya[:, 0:F], ya[:, 2 * F:3 * F], rgb[:, 2 * F:3 * F], OP.max)
                nc.vector.tensor_tensor(ya[:, 2 * F:3 * F], rgb[:, 0:F], rgb[:, F:2 * F], OP.min)
                nc.vector.tensor_tensor(ya[:, F:2 * F], ya[:, 2 * F:3 * F], rgb[:, 2 * F:3 * F], OP.min)
                nc.vector.tensor_tensor(ya[:, 2 * F:3 * F], ya[:, 0:F], ya[:, F:2 * F], OP.subtract)
                # masks: y3[0:F]=(r>=maxc) y3[F:2F]=(g>=maxc); d1 -> y3[2F:3F]
                mxb = ya[:, 0:F][:, None, :].broadcast_to([128, 2, F])
                nc.vector.tensor_tensor(
                    y3[:, 0:2 * F].bitcast(mybir.dt.int16).rearrange("p (c f) -> p c f", c=2),
                    rgb[:, 0:2 * F].rearrange("p (c f) -> p c f", c=2),
                    mxb, OP.is_ge,
                )
                nc.vector.tensor_tensor(y3[:, 2 * F:3 * F], rgb[:, F:2 * F], rgb[:, 2 * F:3 * F], OP.subtract)
                # jg = 2cr + (b - r)
                nc.vector.tensor_tensor(jt[:, 0:F], rgb[:, 2 * F:3 * F], rgb[:, 0:F], OP.subtract)
                nc.vector.tensor_scalar(jt[:, 2 * F:3 * F], ya[:, 2 * F:3 * F], 2.0, None, OP.mult)
                nc.vector.tensor_tensor(jt[:, F:2 * F], jt[:, 2 * F:3 * F], jt[:, 0:F], OP.add)
                # J = 4cr + (r - g), then case overrides
                nc.vector.tensor_tensor(jt[:, 0:F], rgb[:, 0:F], rgb[:, F:2 * F], OP.subtract)
                nc.vector.tensor_scalar(jt[:, 2 * F:3 * F], ya[:, 2 * F:3 * F], 4.0, None, OP.mult)
                Jt = pool.tile([128, F], F16, tag="Jt")
                nc.vector.tensor_tensor(Jt[:], jt[:, 2 * F:3 * F], jt[:, 0:F], OP.add)
                nc.vector.copy_predicated(Jt[:], y3[:, F:2 * F].bitcast(mybir.dt.int16), jt[:, F:2 * F])
                nc.vector.copy_predicated(Jt[:], y3[:, 0:F].bitcast(mybir.dt.int16), y3[:, 2 * F:3 * F])
                # invc = exp(-ln(cr+eps)) on ACT (f32)
                lc = pool.tile([128, F], F32, tag="lc")
                nc.scalar.activation(lc[:], ya[:, 2 * F:3 * F], AF.Ln, bias=1e-30)
                nc.scalar.activation(lc[:], lc[:], AF.Exp, scale=-1.0)
                # i6 = J*invc + 6hf
                nc.vector.tensor_tensor(jt[:, F:2 * F], Jt[:], lc[:], OP.mult)
                nc.vector.tensor_scalar(jt[:, 0:F], jt[:, F:2 * F], col(i, 7), None, OP.add)
                # recon: A1=|i6+a|, A2=|i6+a-6| per channel (ACT)
                A1 = pool.tile([128, F3], F16, tag="A1")
                A2 = pool.tile([128, F3], F16, tag="A2")
                for ci, a in enumerate((3.0, 1.0, -1.0)):
                    nc.scalar.activation(A1[:, ci * F:(ci + 1) * F], jt[:, 0:F], AF.Abs, bias=a)
                    nc.scalar.activation(A2[:, ci * F:(ci + 1) * F], jt[:, 0:F], AF.Abs, bias=a - 6.0)
                nc.vector.tensor_tensor(y3[:], A1[:], A2[:], OP.min)
                nc.scalar.activation(A1[:], y3[:], AF.Relu, bias=-1.0)
                crb = ya[:, 2 * F:3 * F][:, None, :].broadcast_to([128, 3, F])
                mnb = ya[:, F:2 * F][:, None, :].broadcast_to([128, 3, F])
                mxb3 = ya[:, 0:F][:, None, :].broadcast_to([128, 3, F])
                v3 = lambda t: t[:].rearrange("p (c f) -> p c f", c=3)
                nc.vector.tensor_tensor(v3(A2), v3(A1), crb, OP.mult)
                nc.vector.tensor_tensor(v3(A1), v3(A2), mnb, OP.add)
                nc.vector.tensor_tensor(v3(A2), v3(A1), mxb3, OP.min)
                o3 = pool.tile([128, F3], F32, tag="io", bufs=2)
                nc.scalar.activation(o3[:], A2[:], AF.Copy)

                # ---- store ----
                for c in range(3):
                    dma.dma_start(
                        y_h[i, c].flatten().rearrange("(p f) -> p f", p=128),
                        o3[:, c * F : (c + 1) * F],
                    )

    nc.finalize()
    return nc


def _get_nc():
    if "nc" not in _CACHE:
        _CACHE["nc"] = _build()
    return _CACHE["nc"]


def kernel(x, brightness_f, contrast_f, saturation_f, hue_f, num_samples=1, **_):
    x = np.ascontiguousarray(np.asarray(x, dtype=np.float32))
    bf = np.asarray(brightness_f, np.float32)
    cf = np.asarray(contrast_f, np.float32)
    sf = np.asarray(saturation_f, np.float32)
    hf = np.asarray(hue_f, np.float32)
    B = x.shape[0]
    fac = np.stack(
        [
            bf, cf, (1.0 - cf) / np.float32(NPIX), sf,
            GRAY_W[0] * (1.0 - sf), GRAY_W[1] * (1.0 - sf), GRAY_W[2] * (1.0 - sf),
            6.0 * hf,
        ],
        axis=1,
    ).astype(np.float32)

    nc = _get_nc()
    in_maps = [
        {"x": x[k * NIMG:(k + 1) * NIMG], "fac": fac[k * NIMG:(k + 1) * NIMG]}
        for k in range(8)
    ]
    res = run_bass_kernel_spmd(nc, in_maps, core_ids=list(range(8)), trace=TRACE)
    if TRACE:
        _CACHE["last"] = res
    out = np.concatenate([res.results[k]["y"] for k in range(8)], axis=0)
    return out.astype(np.float32)



# revision 10
# speedup vs baseline: 1.6180x; 1.6180x over previous
"""ColorRandomizer Trainium2 kernel: brightness/contrast/saturation/hue on 8 cores.

Data-parallel: 4 images per core, fp16 storage AND fp16 DRAM I/O (host
converts fp32<->fp16; validated absmax ~8e-3 vs fp32 reference).

Per image (engines annotated; work split DVE / ACT / Pool for overlap):
  S1  x1 = min(x*bf, 1)                       DVE TS [3F], in-place
  S2  mean of gray(x1)                        ACT Identity-accum x3 + GPS reduce
  S3  x2r = relu(cf*x1 + delta)               ACT [3F], in-place
  S4  y3_c = min(x2r,1)*sf  (permuted [b|r|g]) DVE TS x3
      gs = sum_c w_c(1-sf)/sf * y3_c          DVE TS+STT+STT
      x3 = clamp01(y3 + gs)                   Pool add x3 + DVE TS clamp
  S5  hue via tent identity, division-free selection:
      maxc (DVE) minc (Pool) cr (DVE) s2=2minc-maxc (Pool)
      masks [r>=maxc | g>=maxc] (DVE is_ge [2F])
      J = case-select(g-b, 2cr+b-r, 4cr+r-g)  DVE + copy_predicated
      i60 = J * recip(cr+2e-5)                ACT Reciprocal + DVE TT
      per ch: B=|i60+(6hf+a-3)| (ACT), m=|B-3| (ACT [3F])
      out = clamp(m,1,2)*cr + s2              DVE TS + TT + TT
      (identity: min(|z|,|z-6|) == ||z-3|-3|; out=minc+cr*clamp(min-1,0,1))
All ACT funcs (Identity/Relu/Reciprocal/Abs) share one act table -> no swaps.
"""
import sys

for _p in ("/opt/trn_rl_repo",):
    if _p not in sys.path:
        sys.path.append(_p)

import numpy as np
from concourse import bass, bacc, mybir, tile, bass_isa
from concourse.bass_utils import run_bass_kernel_spmd

F32 = mybir.dt.float32
F16 = mybir.dt.float16
I16 = mybir.dt.int16
OP = mybir.AluOpType
AF = mybir.ActivationFunctionType

NIMG = 4          # images per core
H, W = 480, 640
NPIX = H * W      # 307200
F = NPIX // 128   # 2400 free elems per partition per plane
F3 = 3 * F
GRAY_W = (0.299, 0.587, 0.114)
NFAC = 12

# sat/tnt strips use permuted channel order [b|r|g] so that
#   [t2|t3] = [b-r | r-g] and [mr|mg] come from single contiguous [2F] ops.
# POS[c] = strip slot of rgb channel c;  slot p holds channel CHAN[p].
POS = (1, 2, 0)   # r->slot1, g->slot2, b->slot0
CHAN = (2, 0, 1)  # slot0=b, slot1=r, slot2=g
# tent offsets a_c per rgb channel: r:3, g:1, b:-1  (bias col = 6hf + a_c - 3)
HB_COL = (7, 8, 9)  # fac cols for r,g,b bias

TRACE = False     # test.py flips this for profiling
_CACHE = {}


def _build():
    nc = bacc.Bacc(None, target_bir_lowering=False)
    x_h = nc.declare_dram_parameter("x", [NIMG, 3, H, W], F16, isOutput=False)
    fac_h = nc.declare_dram_parameter("fac", [NIMG, NFAC], F32, isOutput=False)
    y_h = nc.declare_dram_parameter("y", [NIMG, 3, H, W], F16, isOutput=True)

    dma = nc.sync  # HWDGE

    # activation float biases must exist as const APs
    for v in (2e-5, -3.0, -1.0):
        t = nc.alloc_sbuf_tensor(f"cst-{v}", [128, 1], F32)
        nc.gpsimd.memset(t.ap(), v)
        nc.const_aps.aps[(F32, v)] = t.ap()
    nc.all_engine_barrier()

    with tile.TileContext(nc) as tc:
        with tc.tile_pool(name="p", bufs=1) as pool:
            # broadcast per-image factors to all partitions once
            fac1 = pool.tile([1, NIMG * NFAC], F32)
            dma.dma_start(fac1[:], fac_h[:].flatten()[None, :])
            facb = pool.tile([128, NIMG * NFAC], F32)
            nc.gpsimd.partition_broadcast(facb[:], fac1[:], channels=128)

            def col(i, k):
                return facb[:, i * NFAC + k : i * NFAC + k + 1]

            for i in range(NIMG):
                # ---- load (fp16, per channel to keep partition alignment) ----
                io = pool.tile([128, F3], F16, tag="io", bufs=2)
                for c in range(3):
                    dma.dma_start(
                        io[:, c * F : (c + 1) * F],
                        x_h[i, c].flatten().rearrange("(p f) -> p f", p=128),
                    )

                # ---- S1 brightness, in-place ----
                nc.vector.tensor_scalar(io[:], io[:], col(i, 0), 1.0, OP.mult, OP.min)

                # ---- S2 mean: per-channel sums via ACT accum ----
                cnd = pool.tile([128, F3], F16, tag="cnd", bufs=2)
                sums = pool.tile([128, 4], F32, tag="sums", bufs=2)
                for c in range(3):
                    nc.scalar.activation(
                        cnd[:, c * F : (c + 1) * F],
                        io[:, c * F : (c + 1) * F],
                        AF.Identity, bias=0.0, scale=1.0,
                        accum_out=sums[:, c : c + 1],
                    )
                ws = pool.tile([128, 4], F32, tag="ws", bufs=2)
                nc.vector.tensor_scalar(ws[:, 0:1], sums[:, 0:1], GRAY_W[0], None, OP.mult)
                nc.vector.scalar_tensor_tensor(ws[:, 1:2], sums[:, 1:2], GRAY_W[1], ws[:, 0:1], OP.mult, OP.add)
                nc.vector.scalar_tensor_tensor(ws[:, 2:3], sums[:, 2:3], GRAY_W[2], ws[:, 1:2], OP.mult, OP.add)
                ssum = pool.tile([128, 2], F32, tag="ssum", bufs=2)
                nc.gpsimd.partition_all_reduce(ssum[:, 0:1], ws[:, 2:3], 128, bass_isa.ReduceOp.add)
                # delta = ssum * (1-cf)/NPIX
                nc.vector.tensor_tensor(ssum[:, 1:2], ssum[:, 0:1], col(i, 2), OP.mult)

                # ---- S3 contrast: x2r = relu(cf*x1 + delta), in-place ----
                nc.scalar.activation(io[:], io[:], AF.Relu, bias=ssum[:, 1:2], scale=col(i, 1))

                # ---- S4 saturation ----
                sat = pool.tile([128, F3], F16, tag="sat", bufs=2)
                for c in range(3):
                    p = POS[c]
                    nc.vector.tensor_scalar(
                        sat[:, p * F : (p + 1) * F],
                        io[:, c * F : (c + 1) * F],
                        1.0, col(i, 3), OP.min, OP.mult,
                    )
                sb = sat[:, 0:F]       # b
                sr = sat[:, F:2 * F]   # r
                sg = sat[:, 2 * F:3 * F]  # g
                gs = pool.tile([128, F], F16, tag="gs", bufs=2)
                nc.vector.tensor_scalar(gs[:], sg, col(i, 5), None, OP.mult)
                nc.vector.scalar_tensor_tensor(gs[:], sr, col(i, 4), gs[:], OP.mult, OP.add)
                nc.vector.scalar_tensor_tensor(gs[:], sb, col(i, 6), gs[:], OP.mult, OP.add)
                gsb = gs[:][:, None, :].broadcast_to([128, 3, F])
                nc.vector.tensor_tensor(
                    sat[:].rearrange("p (c f) -> p c f", c=3),
                    sat[:].rearrange("p (c f) -> p c f", c=3),
                    gsb, OP.add,
                )
                nc.vector.tensor_scalar(sat[:], sat[:], 0.0, 1.0, OP.max, OP.min)

                # ---- S5 hue ----
                maxc = pool.tile([128, F], F16, tag="maxc", bufs=2)
                minc = pool.tile([128, F], F16, tag="minc", bufs=2)
                cr = pool.tile([128, F], F16, tag="cr", bufs=2)
                s2 = pool.tile([128, F], F16, tag="s2", bufs=2)
                invc = pool.tile([128, F], F16, tag="invc", bufs=2)
                masks = pool.tile([128, 2 * F], F16, tag="masks", bufs=2)

                nc.vector.tensor_tensor(maxc[:], sg, sb, OP.max)
                nc.vector.tensor_tensor(maxc[:], maxc[:], sr, OP.max)
                nc.vector.tensor_tensor(minc[:], sg, sb, OP.min)
                nc.vector.tensor_tensor(minc[:], minc[:], sr, OP.min)
                nc.vector.tensor_tensor(cr[:], maxc[:], minc[:], OP.subtract)
                nc.vector.scalar_tensor_tensor(s2[:], minc[:], 2.0, maxc[:], OP.mult, OP.subtract)
                # invc = 1/(cr+eps) via exp(-ln); fp16 out is safe (<= 5e4)
                lc = pool.tile([128, F], F32, tag="lc", bufs=2)
                nc.scalar.activation(lc[:], cr[:], AF.Ln, bias=2e-5)
                nc.scalar.activation(invc[:], lc[:], AF.Exp, scale=-1.0)

                mxb = maxc[:][:, None, :].broadcast_to([128, 2, F])
                nc.vector.tensor_tensor(
                    masks[:].bitcast(I16).rearrange("p (c f) -> p c f", c=2),
                    sat[:, F:3 * F].rearrange("p (c f) -> p c f", c=2),
                    mxb, OP.is_ge,
                )
                # candidates: c1 = g-b ; [t2|t3] = [b-r | r-g]
                nc.vector.tensor_tensor(cnd[:, 0:F], sg, sb, OP.subtract)
                nc.vector.tensor_tensor(
                    cnd[:, F:3 * F].rearrange("p (c f) -> p c f", c=2),
                    sat[:, 0:2 * F].rearrange("p (c f) -> p c f", c=2),
                    sat[:, F:3 * F].rearrange("p (c f) -> p c f", c=2),
                    OP.subtract,
                )
                nc.vector.scalar_tensor_tensor(cnd[:, F:2 * F], cr[:], 2.0, cnd[:, F:2 * F], OP.mult, OP.add)
                nc.vector.scalar_tensor_tensor(cnd[:, 2 * F:3 * F], cr[:], 4.0, cnd[:, 2 * F:3 * F], OP.mult, OP.add)
                # J = select into c3 slot: mg -> c2, then mr -> c1
                nc.vector.copy_predicated(cnd[:, 2 * F:3 * F], masks[:, F:2 * F].bitcast(I16), cnd[:, F:2 * F])
                nc.vector.copy_predicated(cnd[:, 2 * F:3 * F], masks[:, 0:F].bitcast(I16), cnd[:, 0:F])
                # i60 = J * invc  (into c1 slot)
                nc.vector.tensor_tensor(cnd[:, 0:F], cnd[:, 2 * F:3 * F], invc[:], OP.mult)

                # tents: B = |i60 + (6hf + a_c - 3)| per slot; m = |B - 3|
                tnt = pool.tile([128, F3], F16, tag="tnt", bufs=2)
                for p in range(3):
                    nc.scalar.activation(
                        tnt[:, p * F : (p + 1) * F], cnd[:, 0:F],
                        AF.Abs, bias=col(i, HB_COL[CHAN[p]]),
                    )
                nc.scalar.activation(tnt[:], tnt[:], AF.Abs, bias=-3.0)
                nc.vector.tensor_scalar(tnt[:], tnt[:], 1.0, 2.0, OP.max, OP.min)
                v3 = lambda t: t[:].rearrange("p (c f) -> p c f", c=3)
                crb = cr[:][:, None, :].broadcast_to([128, 3, F])
                s2b = s2[:][:, None, :].broadcast_to([128, 3, F])
                nc.vector.tensor_tensor(v3(tnt), v3(tnt), crb, OP.mult)
                nc.vector.tensor_tensor(v3(tnt), v3(tnt), s2b, OP.add)

                # ---- store (per slot -> its rgb channel) ----
                for p in range(3):
                    dma.dma_start(
                        y_h[i, CHAN[p]].flatten().rearrange("(p f) -> p f", p=128),
                        tnt[:, p * F : (p + 1) * F],
                    )

    nc.finalize()
    return nc


def _get_nc():
    if "nc" not in _CACHE:
        _CACHE["nc"] = _build()
    return _CACHE["nc"]


def kernel(x, brightness_f, contrast_f, saturation_f, hue_f, num_samples=1, **_):
    x16 = np.ascontiguousarray(np.asarray(x, dtype=np.float16))
    bf = np.asarray(brightness_f, np.float32)
    cf = np.asarray(contrast_f, np.float32)
    sf = np.asarray(saturation_f, np.float32)
    hf = np.asarray(hue_f, np.float32)
    w0, w1, w2 = GRAY_W
    wpp = (1.0 - sf) / sf
    fac = np.stack(
        [
            bf, cf, (1.0 - cf) / np.float32(NPIX), sf,
            w0 * wpp, w1 * wpp, w2 * wpp,
            6.0 * hf + 0.0,   # r: a=3  -> 6hf + a - 3
            6.0 * hf - 2.0,   # g: a=1
            6.0 * hf - 4.0,   # b: a=-1
            np.zeros_like(bf), np.zeros_like(bf),
        ],
        axis=1,
    ).astype(np.float32)

    nc = _get_nc()
    in_maps = [
        {"x": x16[k * NIMG:(k + 1) * NIMG], "fac": fac[k * NIMG:(k + 1) * NIMG]}
        for k in range(8)
    ]
    res = run_bass_kernel_spmd(nc, in_maps, core_ids=list(range(8)), trace=TRACE)
    if TRACE:
        _CACHE["last"] = res
    out = np.concatenate([res.results[k]["y"] for k in range(8)], axis=0)
    return out.astype(np.float32)


# revision 13
# speedup vs baseline: 1.6742x; 1.0347x over previous
"""ColorRandomizer Trainium2 kernel: brightness/contrast/saturation/hue on 8 cores.

Data-parallel: 4 images per core, fp16 storage AND fp16 DRAM I/O (host
converts fp32<->fp16; validated absmax ~8e-3 vs fp32 reference).

Per image (engines annotated; work split DVE / ACT / Pool for overlap):
  S1  x1 = min(x*bf, 1)                       DVE TS [3F], in-place
  S2  mean of gray(x1)                        ACT Identity-accum x3 + GPS reduce
  S3  x2r = relu(cf*x1 + delta)               ACT [3F], in-place
  S4  y3_c = min(x2r,1)*sf  (permuted [b|r|g]) DVE TS x3
      gs = sum_c w_c(1-sf)/sf * y3_c          DVE TS+STT+STT
      x3 = clamp01(y3 + gs)                   Pool add x3 + DVE TS clamp
  S5  hue via tent identity, division-free selection:
      maxc (DVE) minc (Pool) cr (DVE) s2=2minc-maxc (Pool)
      masks [r>=maxc | g>=maxc] (DVE is_ge [2F])
      J = case-select(g-b, 2cr+b-r, 4cr+r-g)  DVE + copy_predicated
      i60 = J * recip(cr+2e-5)                ACT Reciprocal + DVE TT
      per ch: B=|i60+(6hf+a-3)| (ACT), m=|B-3| (ACT [3F])
      out = clamp(m,1,2)*cr + s2              DVE TS + TT + TT
      (identity: min(|z|,|z-6|) == ||z-3|-3|; out=minc+cr*clamp(min-1,0,1))
All ACT funcs (Identity/Relu/Reciprocal/Abs) share one act table -> no swaps.
"""
import sys

for _p in ("/opt/trn_rl_repo",):
    if _p not in sys.path:
        sys.path.append(_p)

import numpy as np
from concourse import bass, bacc, mybir, tile, bass_isa
from concourse.bass_utils import run_bass_kernel_spmd

F32 = mybir.dt.float32
F16 = mybir.dt.float16
I16 = mybir.dt.int16
OP = mybir.AluOpType
AF = mybir.ActivationFunctionType

NIMG = 4          # images per core
H, W = 480, 640
NPIX = H * W      # 307200
F = NPIX // 128   # 2400 free elems per partition per plane
F3 = 3 * F
GRAY_W = (0.299, 0.587, 0.114)
NFAC = 12

# sat/tnt strips use permuted channel order [b|r|g] so that
#   [t2|t3] = [b-r | r-g] and [mr|mg] come from single contiguous [2F] ops.
# POS[c] = strip slot of rgb channel c;  slot p holds channel CHAN[p].
POS = (1, 2, 0)   # r->slot1, g->slot2, b->slot0
CHAN = (2, 0, 1)  # slot0=b, slot1=r, slot2=g
# tent offsets a_c per rgb channel: r:3, g:1, b:-1  (bias col = 6hf + a_c - 3)
HB_COL = (7, 8, 9)  # fac cols for r,g,b bias

TRACE = False     # test.py flips this for profiling
_CACHE = {}


def _build():
    nc = bacc.Bacc(None, target_bir_lowering=False)
    x_h = nc.declare_dram_parameter("x", [NIMG, 3, H, W], F16, isOutput=False)
    fac_h = nc.declare_dram_parameter("fac", [NIMG, NFAC], F32, isOutput=False)
    y_h = nc.declare_dram_parameter("y", [NIMG, 3, H, W], F16, isOutput=True)

    dma = nc.sync  # HWDGE

    # activation float biases must exist as const APs
    for v in (2e-5, -3.0, -1.0):
        t = nc.alloc_sbuf_tensor(f"cst-{v}", [128, 1], F32)
        nc.gpsimd.memset(t.ap(), v)
        nc.const_aps.aps[(F32, v)] = t.ap()
    nc.all_engine_barrier()

    with tile.TileContext(nc) as tc:
        with tc.tile_pool(name="p", bufs=1) as pool:
            # broadcast per-image factors to all partitions once
            fac1 = pool.tile([1, NIMG * NFAC], F32)
            dma.dma_start(fac1[:], fac_h[:].flatten()[None, :])
            facb = pool.tile([128, NIMG * NFAC], F32)
            nc.gpsimd.partition_broadcast(facb[:], fac1[:], channels=128)

            def col(i, k):
                return facb[:, i * NFAC + k : i * NFAC + k + 1]

            def emit_head(i):
                """load + brightness + mean-accum for image i (hoisted one
                image ahead so engines stay busy across the serial chain)."""
                io = pool.tile([128, F3], F16, tag="big", bufs=3)
                for c in range(3):
                    dma.dma_start(
                        io[:, c * F : (c + 1) * F],
                        x_h[i, c].flatten().rearrange("(p f) -> p f", p=128),
                    )
                # S1 brightness, in-place
                nc.vector.tensor_scalar(io[:], io[:], col(i, 0), 1.0, OP.mult, OP.min)
                # S2 mean: per-channel sums via ACT accum (dummy out -> cnd)
                cnd = pool.tile([128, F3], F16, tag="cnd", bufs=2)
                sums = pool.tile([128, 4], F32, tag="sums", bufs=2)
                for c in range(3):
                    nc.scalar.activation(
                        cnd[:, c * F : (c + 1) * F],
                        io[:, c * F : (c + 1) * F],
                        AF.Identity, bias=0.0, scale=1.0,
                        accum_out=sums[:, c : c + 1],
                    )
                return io, cnd, sums

            heads = {0: emit_head(0)}
            for i in range(NIMG):
                if i + 1 < NIMG:
                    heads[i + 1] = emit_head(i + 1)
                io, cnd, sums = heads.pop(i)
                ws = pool.tile([128, 4], F32, tag="ws", bufs=2)
                nc.vector.tensor_scalar(ws[:, 0:1], sums[:, 0:1], GRAY_W[0], None, OP.mult)
                nc.vector.scalar_tensor_tensor(ws[:, 1:2], sums[:, 1:2], GRAY_W[1], ws[:, 0:1], OP.mult, OP.add)
                nc.vector.scalar_tensor_tensor(ws[:, 2:3], sums[:, 2:3], GRAY_W[2], ws[:, 1:2], OP.mult, OP.add)
                ssum = pool.tile([128, 2], F32, tag="ssum", bufs=2)
                nc.gpsimd.partition_all_reduce(ssum[:, 0:1], ws[:, 2:3], 128, bass_isa.ReduceOp.add)
                # delta = ssum * (1-cf)/NPIX
                nc.vector.tensor_tensor(ssum[:, 1:2], ssum[:, 0:1], col(i, 2), OP.mult)

                # ---- S3 contrast: x2r = relu(cf*x1 + delta), in-place ----
                nc.scalar.activation(io[:], io[:], AF.Relu, bias=ssum[:, 1:2], scale=col(i, 1))

                # ---- S4 saturation ----
                sat = pool.tile([128, F3], F16, tag="sat", bufs=2)
                for c in range(3):
                    p = POS[c]
                    nc.vector.tensor_scalar(
                        sat[:, p * F : (p + 1) * F],
                        io[:, c * F : (c + 1) * F],
                        1.0, col(i, 3), OP.min, OP.mult,
                    )
                sb = sat[:, 0:F]       # b
                sr = sat[:, F:2 * F]   # r
                sg = sat[:, 2 * F:3 * F]  # g
                # gs = sum_c w''_c * y3_c  via TS-mults + TT-adds (STT is 1x)
                gm = pool.tile([128, 2 * F], F16, tag="mk", bufs=2)
                gs, gt = gm[:, 0:F], gm[:, F:2 * F]
                nc.vector.tensor_scalar(gs, sg, col(i, 5), None, OP.mult)
                nc.vector.tensor_scalar(gt, sr, col(i, 4), None, OP.mult)
                nc.vector.tensor_tensor(gs, gs, gt, OP.add)
                nc.vector.tensor_scalar(gt, sb, col(i, 6), None, OP.mult)
                nc.vector.tensor_tensor(gs, gs, gt, OP.add)
                gsb = gs[:, None, :].broadcast_to([128, 3, F])
                nc.vector.tensor_tensor(
                    sat[:].rearrange("p (c f) -> p c f", c=3),
                    sat[:].rearrange("p (c f) -> p c f", c=3),
                    gsb, OP.add,
                )
                nc.vector.tensor_scalar(sat[:], sat[:], 0.0, 1.0, OP.max, OP.min)

                # ---- S5 hue ----
                maxc = pool.tile([128, F], F16, tag="maxc", bufs=2)
                minc = pool.tile([128, F], F16, tag="minc", bufs=2)
                cr = pool.tile([128, F], F16, tag="cr", bufs=2)
                s2 = pool.tile([128, F], F16, tag="s2", bufs=2)
                invc = pool.tile([128, F], F16, tag="invc", bufs=2)

                nc.vector.tensor_tensor(maxc[:], sg, sb, OP.max)
                nc.vector.tensor_tensor(maxc[:], maxc[:], sr, OP.max)
                nc.vector.tensor_tensor(minc[:], sg, sb, OP.min)
                nc.vector.tensor_tensor(minc[:], minc[:], sr, OP.min)
                nc.vector.tensor_tensor(cr[:], maxc[:], minc[:], OP.subtract)
                nc.vector.tensor_tensor(s2[:], minc[:], cr[:], OP.subtract)
                # invc = 1/(cr+eps) via exp(-ln); fp16 out is safe (<= 5e4)
                lc = pool.tile([128, F], F32, tag="lc", bufs=2)
                nc.scalar.activation(lc[:], cr[:], AF.Ln, bias=2e-5)
                nc.scalar.activation(invc[:], lc[:], AF.Exp, scale=-1.0)

                masks = pool.tile([128, 2 * F], F16, tag="mk", bufs=2)
                mxb = maxc[:][:, None, :].broadcast_to([128, 2, F])
                nc.vector.tensor_tensor(
                    masks[:].bitcast(I16).rearrange("p (c f) -> p c f", c=2),
                    sat[:, F:3 * F].rearrange("p (c f) -> p c f", c=2),
                    mxb, OP.is_ge,
                )
                # candidates: c1 = g-b ; [t2|t3] = [b-r | r-g]
                nc.vector.tensor_tensor(cnd[:, 0:F], sg, sb, OP.subtract)
                nc.vector.tensor_tensor(
                    cnd[:, F:3 * F].rearrange("p (c f) -> p c f", c=2),
                    sat[:, 0:2 * F].rearrange("p (c f) -> p c f", c=2),
                    sat[:, F:3 * F].rearrange("p (c f) -> p c f", c=2),
                    OP.subtract,
                )
                kk = pool.tile([128, 2 * F], F16, tag="kk", bufs=2)
                nc.vector.tensor_scalar(kk[:, 0:F], cr[:], 2.0, None, OP.mult)
                nc.vector.tensor_scalar(kk[:, F:2 * F], kk[:, 0:F], 2.0, None, OP.mult)
                nc.vector.tensor_tensor(cnd[:, F:2 * F], cnd[:, F:2 * F], kk[:, 0:F], OP.add)
                nc.vector.tensor_tensor(cnd[:, 2 * F:3 * F], cnd[:, 2 * F:3 * F], kk[:, F:2 * F], OP.add)
                # J = select into c3 slot: mg -> c2, then mr -> c1
                nc.vector.copy_predicated(cnd[:, 2 * F:3 * F], masks[:, F:2 * F].bitcast(I16), cnd[:, F:2 * F])
                nc.vector.copy_predicated(cnd[:, 2 * F:3 * F], masks[:, 0:F].bitcast(I16), cnd[:, 0:F])
                # i60 = J * invc  (into c1 slot)
                nc.vector.tensor_tensor(cnd[:, 0:F], cnd[:, 2 * F:3 * F], invc[:], OP.mult)

                # tents: B = |i60 + (6hf + a_c - 3)| per slot; m = |B - 3|
                tnt = pool.tile([128, F3], F16, tag="big", bufs=3)
                for p in range(3):
                    nc.scalar.activation(
                        tnt[:, p * F : (p + 1) * F], cnd[:, 0:F],
                        AF.Abs, bias=col(i, HB_COL[CHAN[p]]),
                    )
                nc.scalar.activation(tnt[:], tnt[:], AF.Abs, bias=-3.0)
                nc.vector.tensor_scalar(tnt[:], tnt[:], 1.0, 2.0, OP.max, OP.min)
                v3 = lambda t: t[:].rearrange("p (c f) -> p c f", c=3)
                crb = cr[:][:, None, :].broadcast_to([128, 3, F])
                s2b = s2[:][:, None, :].broadcast_to([128, 3, F])
                nc.vector.tensor_tensor(v3(tnt), v3(tnt), crb, OP.mult)
                nc.vector.tensor_tensor(v3(tnt), v3(tnt), s2b, OP.add)

                # ---- store (per slot -> its rgb channel) ----
                for p in range(3):
                    dma.dma_start(
                        y_h[i, CHAN[p]].flatten().rearrange("(p f) -> p f", p=128),
                        tnt[:, p * F : (p + 1) * F],
                    )

    nc.finalize()
    return nc


def _get_nc():
    if "nc" not in _CACHE:
        _CACHE["nc"] = _build()
    return _CACHE["nc"]


def kernel(x, brightness_f, contrast_f, saturation_f, hue_f, num_samples=1, **_):
    x16 = np.ascontiguousarray(np.asarray(x, dtype=np.float16))
    bf = np.asarray(brightness_f, np.float32)
    cf = np.asarray(contrast_f, np.float32)
    sf = np.asarray(saturation_f, np.float32)
    hf = np.asarray(hue_f, np.float32)
    w0, w1, w2 = GRAY_W
    wpp = (1.0 - sf) / sf
    fac = np.stack(
        [
            bf, cf, (1.0 - cf) / np.float32(NPIX), sf,
            w0 * wpp, w1 * wpp, w2 * wpp,
            6.0 * hf + 0.0,   # r: a=3  -> 6hf + a - 3
            6.0 * hf - 2.0,   # g: a=1
            6.0 * hf - 4.0,   # b: a=-1
            np.zeros_like(bf), np.zeros_like(bf),
        ],
        axis=1,
    ).astype(np.float32)

    nc = _get_nc()
    in_maps = [
        {"x": x16[k * NIMG:(k + 1) * NIMG], "fac": fac[k * NIMG:(k + 1) * NIMG]}
        for k in range(8)
    ]
    res = run_bass_kernel_spmd(nc, in_maps, core_ids=list(range(8)), trace=TRACE)
    if TRACE:
        _CACHE["last"] = res
    out = np.concatenate([res.results[k]["y"] for k in range(8)], axis=0)
    return out.astype(np.float32)


# revision 18
# speedup vs baseline: 1.7919x; 1.0703x over previous
"""ColorRandomizer Trainium2 kernel: brightness/contrast/saturation/hue on 8 cores.

Data-parallel: 4 images per core, fp16 storage AND fp16 DRAM I/O (host
converts fp32<->fp16; validated absmax ~8e-3 vs fp32 reference).

Per image (engines annotated; work split DVE / ACT / Pool for overlap):
  S1  x1 = min(x*bf, 1)                       DVE TS [3F], in-place
  S2  mean of gray(x1)                        ACT Identity-accum x3 + GPS reduce
  S3  x2r = relu(cf*x1 + delta)               ACT [3F], in-place
  S4  y3_c = min(x2r,1)*sf  (permuted [b|r|g]) DVE TS x3
      gs = sum_c w_c(1-sf)/sf * y3_c          DVE TS+STT+STT
      x3 = clamp01(y3 + gs)                   Pool add x3 + DVE TS clamp
  S5  hue via tent identity, division-free selection:
      maxc (DVE) minc (Pool) cr (DVE) s2=2minc-maxc (Pool)
      masks [r>=maxc | g>=maxc] (DVE is_ge [2F])
      J = case-select(g-b, 2cr+b-r, 4cr+r-g)  DVE + copy_predicated
      i60 = J * recip(cr+2e-5)                ACT Reciprocal + DVE TT
      per ch: B=|i60+(6hf+a-3)| (ACT), m=|B-3| (ACT [3F])
      out = clamp(m,1,2)*cr + s2              DVE TS + TT + TT
      (identity: min(|z|,|z-6|) == ||z-3|-3|; out=minc+cr*clamp(min-1,0,1))
All ACT funcs (Identity/Relu/Reciprocal/Abs) share one act table -> no swaps.
"""
import sys

for _p in ("/opt/trn_rl_repo",):
    if _p not in sys.path:
        sys.path.append(_p)

import numpy as np
from concourse import bass, bacc, mybir, tile, bass_isa
from concourse.bass_utils import run_bass_kernel_spmd

F32 = mybir.dt.float32
F16 = mybir.dt.float16
I16 = mybir.dt.int16
OP = mybir.AluOpType
AF = mybir.ActivationFunctionType

NIMG = 4          # images per core
H, W = 480, 640
NPIX = H * W      # 307200
F = NPIX // 128   # 2400 free elems per partition per plane
F3 = 3 * F
GRAY_W = (0.299, 0.587, 0.114)
NFAC = 12

# sat/tnt strips use permuted channel order [b|r|g] so that
#   [t2|t3] = [b-r | r-g] and [mr|mg] come from single contiguous [2F] ops.
# POS[c] = strip slot of rgb channel c;  slot p holds channel CHAN[p].
POS = (1, 2, 0)   # r->slot1, g->slot2, b->slot0
CHAN = (2, 0, 1)  # slot0=b, slot1=r, slot2=g
# tent offsets a_c per rgb channel: r:3, g:1, b:-1  (bias col = 6hf + a_c - 3)
HB_COL = (7, 8, 9)  # fac cols for r,g,b bias

TRACE = False     # test.py flips this for profiling
_CACHE = {}


def _build():
    nc = bacc.Bacc(None, target_bir_lowering=False)
    x_h = nc.declare_dram_parameter("x", [NIMG, 3, H, W], F16, isOutput=False)
    fac_h = nc.declare_dram_parameter("fac", [NIMG, NFAC], F32, isOutput=False)
    y_h = nc.declare_dram_parameter("y", [NIMG, 3, H, W], F16, isOutput=True)

    dma = nc.sync  # HWDGE

    # activation float biases must exist as const APs
    for v in (2e-5, -3.0, -1.0):
        t = nc.alloc_sbuf_tensor(f"cst-{v}", [128, 1], F32)
        nc.gpsimd.memset(t.ap(), v)
        nc.const_aps.aps[(F32, v)] = t.ap()
    nc.all_engine_barrier()

    with tile.TileContext(nc) as tc:
        with tc.tile_pool(name="p", bufs=1) as pool:
            # broadcast per-image factors to all partitions once
            fac1 = pool.tile([1, NIMG * NFAC], F32)
            dma.dma_start(fac1[:], fac_h[:].flatten()[None, :])
            facb = pool.tile([128, NIMG * NFAC], F32)
            nc.gpsimd.partition_broadcast(facb[:], fac1[:], channels=128)

            def col(i, k):
                return facb[:, i * NFAC + k : i * NFAC + k + 1]

            def emit_head(i):
                """load + brightness + mean-accum for image i (hoisted one
                image ahead so engines stay busy across the serial chain)."""
                io = pool.tile([128, F3], F16, tag="big", bufs=3)
                for c in range(3):
                    dma.dma_start(
                        io[:, c * F : (c + 1) * F],
                        x_h[i, c].flatten().rearrange("(p f) -> p f", p=128),
                    )
                # S1 brightness, in-place
                nc.vector.tensor_scalar(io[:], io[:], col(i, 0), 1.0, OP.mult, OP.min)
                # S2 mean: per-channel sums via ACT accum (dummy out -> cnd)
                cnd = pool.tile([128, F3], F16, tag="cnd", bufs=2)
                sums = pool.tile([128, 4], F32, tag="sums", bufs=2)
                for c in range(3):
                    nc.scalar.activation(
                        cnd[:, c * F : (c + 1) * F],
                        io[:, c * F : (c + 1) * F],
                        AF.Identity, bias=0.0, scale=1.0,
                        accum_out=sums[:, c : c + 1],
                    )
                return io, cnd, sums

            heads = {0: emit_head(0)}
            for i in range(NIMG):
                io, cnd, sums = heads.pop(i)
                ws = pool.tile([128, 4], F32, tag="ws", bufs=2)
                nc.vector.tensor_scalar(ws[:, 0:1], sums[:, 0:1], GRAY_W[0], None, OP.mult)
                nc.vector.scalar_tensor_tensor(ws[:, 1:2], sums[:, 1:2], GRAY_W[1], ws[:, 0:1], OP.mult, OP.add)
                nc.vector.scalar_tensor_tensor(ws[:, 2:3], sums[:, 2:3], GRAY_W[2], ws[:, 1:2], OP.mult, OP.add)
                ssum = pool.tile([128, 2], F32, tag="ssum", bufs=2)
                nc.gpsimd.partition_all_reduce(ssum[:, 0:1], ws[:, 2:3], 128, bass_isa.ReduceOp.add)
                # delta = ssum * (1-cf)/NPIX
                nc.vector.tensor_tensor(ssum[:, 1:2], ssum[:, 0:1], col(i, 2), OP.mult)

                # ---- S3 contrast: x2r = relu(cf*x1 + delta), in-place ----
                nc.scalar.activation(io[:], io[:], AF.Relu, bias=ssum[:, 1:2], scale=col(i, 1))

                # hoist next image's load+S1+accum here (after relu_i so the
                # ACT queue serves relu_i before the next accums)
                if i + 1 < NIMG:
                    heads[i + 1] = emit_head(i + 1)

                # ---- S4 saturation ----
                sat = pool.tile([128, F3], F16, tag="sat", bufs=2)
                for c in range(3):
                    p = POS[c]
                    nc.vector.tensor_scalar(
                        sat[:, p * F : (p + 1) * F],
                        io[:, c * F : (c + 1) * F],
                        1.0, col(i, 3), OP.min, OP.mult,
                    )
                sb = sat[:, 0:F]       # b
                sr = sat[:, F:2 * F]   # r
                sg = sat[:, 2 * F:3 * F]  # g
                # gs = sum_c w''_c * y3_c  via TS-mults + TT-adds (STT is 1x)
                gm = pool.tile([128, 2 * F], F16, tag="mk", bufs=2)
                gs, gt = gm[:, 0:F], gm[:, F:2 * F]
                nc.vector.tensor_scalar(gs, sg, col(i, 5), None, OP.mult)
                nc.vector.tensor_scalar(gt, sr, col(i, 4), None, OP.mult)
                nc.vector.tensor_tensor(gs, gs, gt, OP.add)
                nc.vector.tensor_scalar(gt, sb, col(i, 6), None, OP.mult)
                nc.vector.tensor_tensor(gs, gs, gt, OP.add)
                gsb = gs[:, None, :].broadcast_to([128, 3, F])
                nc.vector.tensor_tensor(
                    sat[:].rearrange("p (c f) -> p c f", c=3),
                    sat[:].rearrange("p (c f) -> p c f", c=3),
                    gsb, OP.add,
                )
                nc.vector.tensor_scalar(sat[:], sat[:], 0.0, 1.0, OP.max, OP.min)

                # ---- S5 hue ----
                maxc = pool.tile([128, F], F16, tag="maxc", bufs=2)
                minc = pool.tile([128, F], F16, tag="minc", bufs=2)
                cr = pool.tile([128, F], F16, tag="cr", bufs=2)
                s2 = pool.tile([128, F], F16, tag="s2", bufs=2)
                invc = pool.tile([128, F], F16, tag="invc", bufs=2)

                nc.vector.tensor_tensor(maxc[:], sg, sb, OP.max)
                nc.vector.tensor_tensor(maxc[:], maxc[:], sr, OP.max)
                nc.vector.tensor_tensor(minc[:], sg, sb, OP.min)
                nc.vector.tensor_tensor(minc[:], minc[:], sr, OP.min)
                nc.vector.tensor_tensor(cr[:], maxc[:], minc[:], OP.subtract)
                nc.vector.tensor_tensor(s2[:], minc[:], cr[:], OP.subtract)
                # invc = 1/(cr+eps) via exp(-ln); fp16 out is safe (<= 5e4)
                lc = pool.tile([128, F], F32, tag="lc", bufs=2)
                nc.scalar.activation(lc[:], cr[:], AF.Ln, bias=2e-5)
                nc.scalar.activation(invc[:], lc[:], AF.Exp, scale=-1.0)

                masks = pool.tile([128, 2 * F], F16, tag="mk", bufs=2)
                mxb = maxc[:][:, None, :].broadcast_to([128, 2, F])
                nc.vector.tensor_tensor(
                    masks[:].bitcast(I16).rearrange("p (c f) -> p c f", c=2),
                    sat[:, F:3 * F].rearrange("p (c f) -> p c f", c=2),
                    mxb, OP.is_ge,
                )
                # candidates: c1 = g-b ; [t2|t3] = [b-r | r-g]
                nc.vector.tensor_tensor(cnd[:, 0:F], sg, sb, OP.subtract)
                nc.vector.tensor_tensor(
                    cnd[:, F:3 * F].rearrange("p (c f) -> p c f", c=2),
                    sat[:, 0:2 * F].rearrange("p (c f) -> p c f", c=2),
                    sat[:, F:3 * F].rearrange("p (c f) -> p c f", c=2),
                    OP.subtract,
                )
                kk = pool.tile([128, 2 * F], F16, tag="kk", bufs=2)
                nc.vector.tensor_scalar(kk[:, 0:F], cr[:], 2.0, None, OP.mult)
                nc.vector.tensor_scalar(kk[:, F:2 * F], kk[:, 0:F], 2.0, None, OP.mult)
                nc.vector.tensor_tensor(cnd[:, F:3 * F], cnd[:, F:3 * F], kk[:], OP.add)
                # J = select into c3 slot: mg -> c2, then mr -> c1
                nc.vector.copy_predicated(cnd[:, 2 * F:3 * F], masks[:, F:2 * F].bitcast(I16), cnd[:, F:2 * F])
                nc.vector.copy_predicated(cnd[:, 2 * F:3 * F], masks[:, 0:F].bitcast(I16), cnd[:, 0:F])
                # i60 = J * invc  (into c1 slot)
                nc.vector.tensor_tensor(cnd[:, 0:F], cnd[:, 2 * F:3 * F], invc[:], OP.mult)

                # tents: B = |i60 + (6hf + a_c - 3)| per slot; m = |B - 3|
                tnt = pool.tile([128, F3], F16, tag="big", bufs=3)
                if i < NIMG - 1:
                    for p in range(3):
                        nc.scalar.activation(
                            tnt[:, p * F : (p + 1) * F], cnd[:, 0:F],
                            AF.Abs, bias=col(i, HB_COL[CHAN[p]]),
                        )
                    nc.scalar.activation(tnt[:], tnt[:], AF.Abs, bias=-3.0)
                    nc.vector.tensor_scalar(tnt[:], tnt[:], 1.0, 2.0, OP.max, OP.min)
                    v3 = lambda t: t[:].rearrange("p (c f) -> p c f", c=3)
                    crb = cr[:][:, None, :].broadcast_to([128, 3, F])
                    s2b = s2[:][:, None, :].broadcast_to([128, 3, F])
                    nc.vector.tensor_tensor(v3(tnt), v3(tnt), crb, OP.mult)
                    nc.vector.tensor_tensor(v3(tnt), v3(tnt), s2b, OP.add)
                    for p in range(3):
                        dma.dma_start(
                            y_h[i, CHAN[p]].flatten().rearrange("(p f) -> p f", p=128),
                            tnt[:, p * F : (p + 1) * F],
                        )
                else:
                    # last image: per-channel tail so ACT(B,m) overlaps DVE(t,q,out)
                    for p in range(3):
                        sl = tnt[:, p * F : (p + 1) * F]
                        nc.scalar.activation(sl, cnd[:, 0:F], AF.Abs, bias=col(i, HB_COL[CHAN[p]]))
                        nc.scalar.activation(sl, sl, AF.Abs, bias=-3.0)
                        nc.vector.tensor_scalar(sl, sl, 1.0, 2.0, OP.max, OP.min)
                        nc.vector.tensor_tensor(sl, sl, cr[:], OP.mult)
                        nc.vector.tensor_tensor(sl, sl, s2[:], OP.add)
                        dma.dma_start(
                            y_h[i, CHAN[p]].flatten().rearrange("(p f) -> p f", p=128),
                            sl,
                        )

    nc.finalize()
    return nc


def _get_nc():
    if "nc" not in _CACHE:
        _CACHE["nc"] = _build()
    return _CACHE["nc"]


def kernel(x, brightness_f, contrast_f, saturation_f, hue_f, num_samples=1, **_):
    x16 = np.ascontiguousarray(np.asarray(x, dtype=np.float16))
    bf = np.asarray(brightness_f, np.float32)
    cf = np.asarray(contrast_f, np.float32)
    sf = np.asarray(saturation_f, np.float32)
    hf = np.asarray(hue_f, np.float32)
    w0, w1, w2 = GRAY_W
    wpp = (1.0 - sf) / sf
    fac = np.stack(
        [
            bf, cf, (1.0 - cf) / np.float32(NPIX), sf,
            w0 * wpp, w1 * wpp, w2 * wpp,
            6.0 * hf + 0.0,   # r: a=3  -> 6hf + a - 3
            6.0 * hf - 2.0,   # g: a=1
            6.0 * hf - 4.0,   # b: a=-1
            np.zeros_like(bf), np.zeros_like(bf),
        ],
        axis=1,
    ).astype(np.float32)

    nc = _get_nc()
    in_maps = [
        {"x": x16[k * NIMG:(k + 1) * NIMG], "fac": fac[k * NIMG:(k + 1) * NIMG]}
        for k in range(8)
    ]
    res = run_bass_kernel_spmd(nc, in_maps, core_ids=list(range(8)), trace=TRACE)
    if TRACE:
        _CACHE["last"] = res
    out = np.concatenate([res.results[k]["y"] for k in range(8)], axis=0)
    return out.astype(np.float32)
